# revision 21
# baseline (speedup 1.0000x reference)
"""CopyDecoder Trainium2 kernel (nn_CopyDecoder_5274219840242).

Sharding: 8 cores = 4 batches x 2 query-halves (data parallel, no collectives).

Per core (b, q-slab of 256 rows):
  - attention: Q/K projections with the fcQ stage folded into Wq on the host
    (Q = (Wq@WfcQ) @ dec.T + (Wq@bfcQ + bq)); per-head softmax (logits
    bounded, no max-subtraction), head mean.
  - duplicate-combining selection matrix Dm[s,s'] = [src_s == src_s'];
    a_comb = attn @ Dm; e = exp(a_comb/NH).
  - denom[q] = V + sum_s (e[q,s]-1)/cnt[s] (softmax denominator over vocab,
    exploiting exp(0)=1 for untouched vocab entries).
  - streaming blend over p1 in bf16 both directions (the tolerance is 2e-2
    and the blend is error-linear in p1, so bf16's 0.2% rounding is safe;
    halves the DMA-engine traffic, which is the roofline here):
    out = s1*p1 + s2 with per-partition scalars, all blends on DVE.
  - w = sigmoid(dec @ Wfcw.T + b) is computed on the HOST in fp32 (tiny,
    and w needs fp32: its error is amplified ~|p2-p1|/out); s1 = 1-w ships
    as a constant. s2 = w/denom is computed on device.
  - fix values for the <=512 source-token columns: fix = s1*p1c + s2*e,
    written as a bf16 side output; the host scatters them into the final
    fp32 output during unshard.

Scheduling (the DMA engines are the bottleneck: 16 engines shared by the
two HWDGE queues, ~26 GB/s each, full duplex read+write):
  - consts ride two host-prepacked bf16 buffers with 8KB-contiguous
    partition lines (no 1KB-packet storms).
  - sync queue: pk1, pk2, then the p1-in stream (12-deep prefetch), plus
    the first three out-stores (so writes start the moment s2[0] exists
    without putting a stall in front of chain-1's activations).
  - scalar queue: small consts, then attention-chain ACT ops for BOTH
    mi chains, then the remaining out-stores.
  - emission order keeps each engine's in-order stream hazard-free:
    chain0, 3 early blends (out on sync), chain1, remaining blends.
"""

import sys

sys.path.insert(0, "/opt/trn_rl_repo")

import numpy as np

import concourse.bacc as bacc
import concourse.bass as bass
import concourse.mybir as mybir
import concourse.tile as tile
from concourse.bass_utils import run_bass_kernel_spmd
from concourse.masks import make_identity

P = 128
D = 512
TS = 512
TQH = 256  # q rows per core
V = 32000
NH = 8
DH = 64
KC = D // P  # 4 contraction chunks
MI = TQH // P  # 2 q partition tiles
SC = TS // P  # 4 source-position chunks
VT = 4000  # vocab columns per blend tile (8KB bf16 DMA lines)
NVT = V // VT  # 8 vocab tiles per q partition tile

F32 = mybir.dt.float32
BF16 = mybir.dt.bfloat16
I32 = mybir.dt.int32
AF = mybir.ActivationFunctionType
ALU = mybir.AluOpType
AX = mybir.AxisListType

# pk1 layout (bf16, per-partition cols): [decTb 4*256 | wqf 4*512]
PK1_DEC = 0
PK1_WQF = KC * TQH  # 1024
PK1_COLS = PK1_WQF + KC * D  # 3072
# pk2 layout: [wkb 4*512 | encb 4*512 | p1cb 2*512]
PK2_WKB = 0
PK2_ENC = KC * D  # 2048
PK2_P1C = 2 * KC * D  # 4096
PK2_COLS = PK2_P1C + MI * TS  # 5120
# smalls layout (f32): [bqf 4 | bk 4 | w 2 | s1 2]
SM_BQF = 0
SM_BK = KC
SM_W = 2 * KC
SM_S1 = 2 * KC + MI
SM_COLS = 2 * KC + 2 * MI  # 12

_NC_CACHE = None
_LAST_RESULTS = None


def build_nc():
    nc = bacc.Bacc("TRN2", target_bir_lowering=False, debug=False)

    pk1 = nc.dram_tensor("pk1", [P, PK1_COLS], BF16, kind="ExternalInput")
    pk2 = nc.dram_tensor("pk2", [P, PK2_COLS], BF16, kind="ExternalInput")
    smalls = nc.dram_tensor("smalls", [P, SM_COLS], F32, kind="ExternalInput")
    src = nc.dram_tensor("src", [TS, 1], I32, kind="ExternalInput")
    p1 = nc.dram_tensor("p1", [TQH, V], BF16, kind="ExternalInput")
    out = nc.dram_tensor("out", [TQH, V], BF16, kind="ExternalOutput")
    fixc = nc.dram_tensor("fixc", [TQH, TS], BF16, kind="ExternalOutput")

    with tile.TileContext(nc) as tc:
        with (
            tc.tile_pool(name="const", bufs=1) as cp,
            tc.tile_pool(name="work", bufs=5) as wp,
            tc.tile_pool(name="pin", bufs=13) as pinp,
            tc.tile_pool(name="pout", bufs=4) as poutp,
            tc.tile_pool(name="ps", bufs=8, space="PSUM") as psp,
        ):
            # ---- persistent SBUF tiles ----
            pk1_sb = cp.tile([P, PK1_COLS], BF16, tag="pk1_sb")
            pk2_sb = cp.tile([P, PK2_COLS], BF16, tag="pk2_sb")
            smalls_sb = cp.tile([P, SM_COLS], F32, tag="smalls_sb")
            src_sb = cp.tile([P, SC], I32, tag="src_sb")
            srcf_sb = cp.tile([P, SC], F32, tag="srcf_sb")
            ident_sb = cp.tile([P, P], F32, tag="ident_sb")
            identb_sb = cp.tile([P, P], BF16, tag="identb_sb")
            srcrow_sb = cp.tile([P, TS], F32, tag="srcrow_sb")
            invcntrow_sb = cp.tile([P, TS], F32, tag="invcntrow_sb")
            Dm_sb = cp.tile([P, SC, TS], BF16, tag="Dm_sb")
            cnt_sb = cp.tile([P, SC], F32, tag="cnt_sb")
            invcnt_sb = cp.tile([P, SC], F32, tag="invcnt_sb")
            qTb_sb = cp.tile([P, KC, TQH], BF16, tag="qTb_sb")
            kTb_sb = cp.tile([P, KC, TS], BF16, tag="kTb_sb")
            attn_sb = cp.tile([P, MI, TS], BF16, tag="attn_sb")
            attnT_sb = cp.tile([P, SC, TQH], BF16, tag="attnT_sb")
            e_sb = cp.tile([P, MI, TS], F32, tag="e_sb")
            sume_sb = cp.tile([P, MI], F32, tag="sume_sb")
            denom_sb = cp.tile([P, MI], F32, tag="denom_sb")
            rden_sb = cp.tile([P, MI], F32, tag="rden_sb")
            s2_sb = cp.tile([P, MI], F32, tag="s2_sb")

            def wqf_c(kc, mc):
                o = PK1_WQF + kc * D + mc * P
                return pk1_sb[:, o : o + P]

            def decTb_c(kc):
                o = PK1_DEC + kc * TQH
                return pk1_sb[:, o : o + TQH]

            def wkb_c(kc, mc):
                o = PK2_WKB + kc * D + mc * P
                return pk2_sb[:, o : o + P]

            def encb_c(kc):
                o = PK2_ENC + kc * D
                return pk2_sb[:, o : o + D]

            def p1cb_v(mi):
                o = PK2_P1C + mi * TS
                return pk2_sb[:, o : o + TS]

            def s1_col(mi):
                return smalls_sb[:, SM_S1 + mi : SM_S1 + mi + 1]

            def w_col(mi):
                return smalls_sb[:, SM_W + mi : SM_W + mi + 1]

            # ---- loads (src rides sync first: tiny, and the Dm build needs
            #      it before the scalar queue's small packets would drain
            #      behind pk1/pk2's 8KB streams; pk loads are split per-kc
            #      chunk so the first Q matmul starts ~4us earlier) ----
            nc.sync.dma_start(
                out=src_sb[:], in_=src[:].rearrange("(c p) n -> p (c n)", p=P)
            )
            nc.scalar.dma_start(out=smalls_sb[:], in_=smalls[:])
            for kc in range(KC):
                nc.sync.dma_start(
                    out=pk1_sb[:, kc * TQH : (kc + 1) * TQH],
                    in_=pk1[:, kc * TQH : (kc + 1) * TQH],
                )
                nc.scalar.dma_start(
                    out=pk1_sb[:, PK1_WQF + kc * D : PK1_WQF + (kc + 1) * D],
                    in_=pk1[:, PK1_WQF + kc * D : PK1_WQF + (kc + 1) * D],
                )
            for kc in range(KC):
                nc.sync.dma_start(
                    out=pk2_sb[:, kc * D : (kc + 1) * D],
                    in_=pk2[:, kc * D : (kc + 1) * D],
                )
                nc.scalar.dma_start(
                    out=pk2_sb[:, PK2_ENC + kc * D : PK2_ENC + (kc + 1) * D],
                    in_=pk2[:, PK2_ENC + kc * D : PK2_ENC + (kc + 1) * D],
                )
            nc.sync.dma_start(
                out=pk2_sb[:, PK2_P1C:PK2_COLS], in_=pk2[:, PK2_P1C:PK2_COLS]
            )
            nc.vector.tensor_copy(srcf_sb[:], src_sb[:])
            make_identity(nc, ident_sb[:])
            make_identity(nc, identb_sb[:])

            # ---- Q_T = Wqf @ dec.T + bqf  (fcQ folded on host; emitted
            #      first so PE starts the moment pk1 lands) ----
            for mc in range(KC):
                ps = psp.tile([P, TQH], F32, tag="ps")
                for kc in range(KC):
                    nc.tensor.matmul(
                        out=ps[:],
                        lhsT=wqf_c(kc, mc),
                        rhs=decTb_c(kc),
                        start=(kc == 0),
                        stop=(kc == KC - 1),
                    )
                nc.scalar.activation(
                    qTb_sb[:, mc, :], ps[:], AF.Identity,
                    bias=smalls_sb[:, SM_BQF + mc : SM_BQF + mc + 1], scale=1.0,
                )

            # ---- selection matrix Dm, counts (PE transposes slot between
            #      Q and K; Dm itself is only needed ~10us later) ----
            for c in range(SC):
                pt = psp.tile([P, P], F32, tag="ps")
                nc.tensor.transpose(
                    out=pt[:],
                    in_=srcf_sb[:, c : c + 1].to_broadcast([P, P]),
                    identity=ident_sb[:],
                )
                nc.vector.tensor_copy(srcrow_sb[:, c * P : (c + 1) * P], pt[:])
            for a in range(SC):
                nc.vector.tensor_tensor(
                    out=Dm_sb[:, a, :],
                    in0=srcf_sb[:, a : a + 1].to_broadcast([P, TS]),
                    in1=srcrow_sb[:],
                    op=ALU.is_equal,
                )
                nc.vector.tensor_reduce(
                    cnt_sb[:, a : a + 1], Dm_sb[:, a, :], AX.X, ALU.add
                )
            nc.vector.reciprocal(invcnt_sb[:], cnt_sb[:])

            # ---- K_T = Wk @ enc.T + bk ----
            for mc in range(KC):
                ps = psp.tile([P, TS], F32, tag="ps")
                for kc in range(KC):
                    nc.tensor.matmul(
                        out=ps[:],
                        lhsT=wkb_c(kc, mc),
                        rhs=encb_c(kc),
                        start=(kc == 0),
                        stop=(kc == KC - 1),
                    )
                nc.scalar.activation(
                    kTb_sb[:, mc, :], ps[:], AF.Identity,
                    bias=smalls_sb[:, SM_BK + mc : SM_BK + mc + 1], scale=1.0,
                )

            # ---- invcnt row layout (PE transposes; off the critical path,
            #      needed only for g) ----
            for c in range(SC):
                pt = psp.tile([P, P], F32, tag="ps")
                nc.tensor.transpose(
                    out=pt[:],
                    in_=invcnt_sb[:, c : c + 1].to_broadcast([P, P]),
                    identity=ident_sb[:],
                )
                nc.vector.tensor_copy(invcntrow_sb[:, c * P : (c + 1) * P], pt[:])

            p1_v = p1[:].rearrange("(mi p) v -> p mi v", p=P)
            out_v = out[:].rearrange("(mi p) v -> p mi v", p=P)

            # pre-issue every p1 tile load, alternating between the two
            # HWDGE queues: a single queue's reads run ~20 GB/s per DMA
            # engine, two active queues interleave to ~26 (measured); the
            # pool dependency (pin N reuses pin N-13's buffer, freed by
            # blend N-13) paces the tail reads
            pins = []
            for mi in range(MI):
                for vt in range(NVT):
                    i = mi * NVT + vt
                    pin = pinp.tile([P, VT], BF16, tag="pin")
                    # pins 13+ wait on pool buffers freed by blends; keep
                    # them off the scalar queue so they can never sit ahead
                    # of the chain ACT ops in that engine's stream
                    eng = nc.scalar if (i % 2 == 1 and i < 12) else nc.sync
                    eng.dma_start(
                        out=pin[:], in_=p1_v[:, mi, vt * VT : (vt + 1) * VT]
                    )
                    pins.append(pin)

            def blend_tile(mi, vt, out_engine):
                vs = slice(vt * VT, (vt + 1) * VT)
                pin = pins[mi * NVT + vt]
                pout = poutp.tile([P, VT], BF16, tag="pout")
                nc.vector.tensor_scalar(
                    out=pout[:], in0=pin[:],
                    scalar1=s1_col(mi),
                    scalar2=s2_sb[:, mi : mi + 1],
                    op0=ALU.mult, op1=ALU.add,
                )
                out_engine.dma_start(out=out_v[:, mi, vs], in_=pout[:])

            def attn_heads(mi):
                # scores + per-head softmax (no max subtraction: |logit| is a
                # ~N(0,1) sample, exp is safe in fp32); accumulate the sum of
                # per-head softmaxes (the 1/NH head-mean folds into the
                # e = exp(a_comb/NH) scale below)
                for h in range(NH):
                    hc, hp = h // 2, h % 2
                    ps = psp.tile([P, TS], F32, tag="ps")
                    nc.tensor.matmul(
                        out=ps[:],
                        lhsT=qTb_sb[hp * DH : (hp + 1) * DH, hc, mi * P : (mi + 1) * P],
                        rhs=kTb_sb[hp * DH : (hp + 1) * DH, hc, :],
                        start=True,
                        stop=True,
                    )
                    ex = wp.tile([P, TS], BF16, tag="ex")
                    se = wp.tile([P, 1], F32, tag="se")
                    nc.scalar.activation(
                        ex[:], ps[:], AF.Exp,
                        bias=0.0, scale=0.125, accum_out=se[:, 0:1],
                    )
                    r8 = wp.tile([P, 1], F32, tag="r8")
                    nc.vector.reciprocal(r8[:], se[:, 0:1])
                    if h == 0:
                        nc.vector.tensor_scalar_mul(attn_sb[:, mi, :], ex[:], r8[:, 0:1])
                    else:
                        nc.vector.scalar_tensor_tensor(
                            out=attn_sb[:, mi, :],
                            in0=ex[:],
                            scalar=r8[:, 0:1],
                            in1=attn_sb[:, mi, :],
                            op0=ALU.mult,
                            op1=ALU.add,
                        )

            def attn_tail(mi):
                # attn_T via PE transpose (for the a_comb contraction)
                for sc in range(SC):
                    pt = psp.tile([P, P], BF16, tag="ps")
                    nc.tensor.transpose(
                        out=pt[:],
                        in_=attn_sb[:, mi, sc * P : (sc + 1) * P],
                        identity=identb_sb[:],
                    )
                    nc.vector.tensor_copy(attnT_sb[:, sc, mi * P : (mi + 1) * P], pt[:])

                # a_comb = attn @ Dm ; e = exp(a_comb/NH) ; denom ; s2
                ps = psp.tile([P, TS], F32, tag="ps")
                for c in range(SC):
                    nc.tensor.matmul(
                        out=ps[:],
                        lhsT=attnT_sb[:, c, mi * P : (mi + 1) * P],
                        rhs=Dm_sb[:, c, :],
                        start=(c == 0),
                        stop=(c == SC - 1),
                    )
                nc.scalar.activation(
                    e_sb[:, mi, :], ps[:], AF.Exp, bias=0.0, scale=1.0 / NH
                )
                g = wp.tile([P, TS], F32, tag="g")
                nc.vector.scalar_tensor_tensor(
                    out=g[:],
                    in0=e_sb[:, mi, :],
                    scalar=-1.0,
                    in1=invcntrow_sb[:],
                    op0=ALU.add,
                    op1=ALU.mult,
                )
                nc.vector.tensor_reduce(sume_sb[:, mi : mi + 1], g[:], AX.X, ALU.add)
                nc.vector.tensor_scalar_add(
                    denom_sb[:, mi : mi + 1], sume_sb[:, mi : mi + 1], float(V)
                )
                nc.vector.reciprocal(rden_sb[:, mi : mi + 1], denom_sb[:, mi : mi + 1])
                nc.vector.tensor_tensor(
                    out=s2_sb[:, mi : mi + 1], in0=w_col(mi),
                    in1=rden_sb[:, mi : mi + 1], op=ALU.mult,
                )

            # both chains fully before any blend, with the head loops
            # interleaved ahead of the tails: chain1's exps run on ACT right
            # behind chain0's instead of waiting for chain0's whole tail,
            # landing s2[0] and s2[1] within ~2us of each other.
            attn_heads(0)
            attn_heads(1)
            attn_tail(0)
            attn_tail(1)

            # ---- fix columns early (off the kernel tail):
            #      fix = s1*p1c + s2*e (bf16 side output) ----
            for mi in range(MI):
                t2 = wp.tile([P, TS], F32, tag="fix_t2")
                nc.vector.tensor_scalar_mul(t2[:], e_sb[:, mi, :], s2_sb[:, mi : mi + 1])
                fb = wp.tile([P, TS], BF16, tag="fix_fb")
                nc.vector.scalar_tensor_tensor(
                    out=fb[:], in0=p1cb_v(mi), scalar=s1_col(mi), in1=t2[:],
                    op0=ALU.mult, op1=ALU.add,
                )
                nc.scalar.dma_start(
                    out=fixc[:].rearrange("(mi p) s -> p mi s", p=P)[:, mi, :],
                    in_=fb[:],
                )

            # force the blends after every chain op in each engine's stream:
            # the scheduler otherwise interleaves them ahead of chain1's DVE
            # tail, and a pout-stalled blend then blocks s2[1] by ~15us
            for mi in range(MI):
                for vt in range(NVT):
                    with tc.tile_wait_until(1.0 + 0.01 * (mi * NVT + vt)):
                        blend_tile(mi, vt, nc.scalar)

    nc.finalize()
    return nc


def _get_nc():
    global _NC_CACHE
    if _NC_CACHE is None:
        _NC_CACHE = build_nc()
    return _NC_CACHE


def _pack_kc(m):
    # [D, cols] -> [P, KC*cols] with row r = kc*P + p at cols [kc*cols ...)
    d, cols = m.shape
    return np.ascontiguousarray(
        m.reshape(KC, P, cols).transpose(1, 0, 2).reshape(P, KC * cols)
    )


def _pack_mi(m):
    # [TQH, cols] -> [P, MI*cols]
    _, cols = m.shape
    return np.ascontiguousarray(
        m.reshape(MI, P, cols).transpose(1, 0, 2).reshape(P, MI * cols)
    )


def kernel(**inputs) -> np.ndarray:
    dec = np.asarray(inputs["dec_output"], dtype=np.float32)  # [4, 512, 512]
    enc = np.asarray(inputs["enc_output"], dtype=np.float32)  # [4, 512, 512]
    src = np.asarray(inputs["src"]).astype(np.int32)  # [4, 512]
    p1 = np.asarray(inputs["p1"], dtype=np.float32)  # [4, 512, 32000]
    WfcQ = np.asarray(inputs["WfcQ"], dtype=np.float32)
    bfcQ = np.asarray(inputs["bfcQ"], dtype=np.float32)
    Wq = np.asarray(inputs["Wq"], dtype=np.float32)
    bq = np.asarray(inputs["bq"], dtype=np.float32)
    Wk = np.asarray(inputs["Wk"], dtype=np.float32)
    bk = np.asarray(inputs["bk"], dtype=np.float32)
    Wfcw = np.asarray(inputs["Wfcw"], dtype=np.float32)
    bfcw = np.asarray(inputs["bfcw"], dtype=np.float32)

    B, TQ, _ = dec.shape
    n_cores = 8

    import ml_dtypes

    bf16 = ml_dtypes.bfloat16

    # fold fcQ into the query projection; gate w computed on host in fp32
    WqfT = np.ascontiguousarray((Wq @ WfcQ).T.astype(bf16))
    bqf = Wq @ bfcQ + bq
    w_all = 1.0 / (1.0 + np.exp(-(dec @ Wfcw[0] + bfcw[0])))  # [4, 512] f32

    wqf_pk = _pack_kc(WqfT)
    wkb_pk = _pack_kc(np.ascontiguousarray(Wk.T.astype(bf16)))
    bqf_pk = bqf.reshape(KC, P).T
    bk_pk = bk.reshape(KC, P).T

    in_maps = []
    for core in range(n_cores):
        b, qh = core // 2, core % 2
        qs = slice(qh * TQH, (qh + 1) * TQH)
        p1_slab = p1[b, qs, :]
        p1c = p1_slab[:, src[b]]  # [TQH, TS] f32 host gather
        pk1 = np.concatenate(
            [_pack_kc(np.ascontiguousarray(dec[b].T[:, qs].astype(bf16))), wqf_pk],
            axis=1,
        )
        pk2 = np.concatenate(
            [
                wkb_pk,
                _pack_kc(np.ascontiguousarray(enc[b].T.astype(bf16))),
                _pack_mi(p1c.astype(bf16)),
            ],
            axis=1,
        )
        w_core = w_all[b, qs].reshape(MI, P).T  # [P, MI]
        smalls = np.concatenate(
            [bqf_pk, bk_pk, w_core, 1.0 - w_core], axis=1
        ).astype(np.float32)
        in_maps.append(
            {
                "pk1": np.ascontiguousarray(pk1),
                "pk2": np.ascontiguousarray(pk2),
                "smalls": np.ascontiguousarray(smalls),
                "src": np.ascontiguousarray(src[b].reshape(TS, 1)),
                "p1": np.ascontiguousarray(p1_slab.astype(bf16)),
            }
        )

    nc = _get_nc()
    res = run_bass_kernel_spmd(nc, in_maps, core_ids=list(range(n_cores)))
    global _LAST_RESULTS
    _LAST_RESULTS = res

    out = np.empty((B, TQ, V), dtype=np.float32)
    for core in range(n_cores):
        b, qh = core // 2, core % 2
        qs = slice(qh * TQH, (qh + 1) * TQH)
        out[b, qs, :] = np.asarray(res.results[core]["out"]).astype(np.float32)
        # place the corrected source-token columns (duplicates carry
        # identical values, so overwrite order does not matter)
        out[b, qs, :][:, src[b]] = np.asarray(res.results[core]["fixc"]).astype(
            np.float32
        )
    return out


# revision 23
# speedup vs baseline: 1.0954x; 1.0954x over previous
"""CopyDecoder Trainium2 kernel (nn_CopyDecoder_5274219840242).

Sharding: 8 cores = 4 batches x 2 query-halves (data parallel, no collectives).

Per core (b, q-slab of 256 rows):
  - attention: Q/K projections with the fcQ stage folded into Wq on the host
    (Q = (Wq@WfcQ) @ dec.T + (Wq@bfcQ + bq)); per-head softmax (logits
    bounded, no max-subtraction), head mean.
  - duplicate-combining selection matrix Dm[s,s'] = [src_s == src_s'];
    a_comb = attn @ Dm; e = exp(a_comb/NH).
  - denom[q] = V + sum_s (e[q,s]-1)/cnt[s] (softmax denominator over vocab,
    exploiting exp(0)=1 for untouched vocab entries).
  - streaming blend over p1 in bf16 both directions (the tolerance is 2e-2
    and the blend is error-linear in p1, so bf16's 0.2% rounding is safe;
    halves the DMA-engine traffic, which is the roofline here):
    out = s1*p1 + s2 with per-partition scalars, all blends on DVE.
  - w = sigmoid(dec @ Wfcw.T + b) is computed on the HOST in fp32 (tiny,
    and w needs fp32: its error is amplified ~|p2-p1|/out); s1 = 1-w ships
    as a constant. s2 = w/denom is computed on device.
  - fix values for the <=512 source-token columns: fix = s1*p1c + s2*e,
    written as a bf16 side output; the host scatters them into the final
    fp32 output during unshard.

Scheduling (the DMA engines are the bottleneck: 16 engines shared by the
two HWDGE queues, ~26 GB/s each, full duplex read+write):
  - consts ride two host-prepacked bf16 buffers with 8KB-contiguous
    partition lines (no 1KB-packet storms).
  - sync queue: pk1, pk2, then the p1-in stream (12-deep prefetch), plus
    the first three out-stores (so writes start the moment s2[0] exists
    without putting a stall in front of chain-1's activations).
  - scalar queue: small consts, then attention-chain ACT ops for BOTH
    mi chains, then the remaining out-stores.
  - emission order keeps each engine's in-order stream hazard-free:
    chain0, 3 early blends (out on sync), chain1, remaining blends.
"""

import sys

sys.path.insert(0, "/opt/trn_rl_repo")

import numpy as np

import concourse.bacc as bacc
import concourse.bass as bass
import concourse.mybir as mybir
import concourse.tile as tile
from concourse.bass_utils import run_bass_kernel_spmd
from concourse.masks import make_identity

P = 128
D = 512
TS = 512
TQH = 256  # q rows per core
V = 32000
NH = 8
DH = 64
KC = D // P  # 4 contraction chunks
MI = TQH // P  # 2 q partition tiles
SC = TS // P  # 4 source-position chunks
VT = 4000  # vocab columns per blend tile (8KB bf16 DMA lines)
NVT = V // VT  # 8 vocab tiles per q partition tile

F32 = mybir.dt.float32
BF16 = mybir.dt.bfloat16
I32 = mybir.dt.int32
AF = mybir.ActivationFunctionType
ALU = mybir.AluOpType
AX = mybir.AxisListType

# pk1 layout (bf16, per-partition cols): [decTb 4*256 | wqf 4*512]
PK1_DEC = 0
PK1_WQF = KC * TQH  # 1024
PK1_COLS = PK1_WQF + KC * D  # 3072
# pk2 layout: [wkb 4*512 | encb 4*512 | p1cb 2*512]
PK2_WKB = 0
PK2_ENC = KC * D  # 2048
PK2_P1C = 2 * KC * D  # 4096
PK2_COLS = PK2_P1C + MI * TS  # 5120
# smalls layout (f32): [bqf 4 | bk 4 | w 2 | s1 2]
SM_BQF = 0
SM_BK = KC
SM_W = 2 * KC
SM_S1 = 2 * KC + MI
SM_COLS = 2 * KC + 2 * MI  # 12

_NC_CACHE = None
_LAST_RESULTS = None


def build_nc():
    nc = bacc.Bacc("TRN2", target_bir_lowering=False, debug=False)

    pk1 = nc.dram_tensor("pk1", [P, PK1_COLS], BF16, kind="ExternalInput")
    pk2 = nc.dram_tensor("pk2", [P, PK2_COLS], BF16, kind="ExternalInput")
    smalls = nc.dram_tensor("smalls", [P, SM_COLS], F32, kind="ExternalInput")
    src = nc.dram_tensor("src", [TS, 1], I32, kind="ExternalInput")
    p1 = nc.dram_tensor("p1", [TQH, V], BF16, kind="ExternalInput")
    out = nc.dram_tensor("out", [TQH, V], BF16, kind="ExternalOutput")
    fixc = nc.dram_tensor("fixc", [TQH, TS], BF16, kind="ExternalOutput")

    with tile.TileContext(nc) as tc:
        with (
            tc.tile_pool(name="const", bufs=1) as cp,
            tc.tile_pool(name="work", bufs=5) as wp,
            tc.tile_pool(name="pin", bufs=13) as pinp,
            tc.tile_pool(name="pout", bufs=4) as poutp,
            tc.tile_pool(name="ps", bufs=8, space="PSUM") as psp,
        ):
            # ---- persistent SBUF tiles ----
            pk1_sb = cp.tile([P, PK1_COLS], BF16, tag="pk1_sb")
            pk2_sb = cp.tile([P, PK2_COLS], BF16, tag="pk2_sb")
            smalls_sb = cp.tile([P, SM_COLS], F32, tag="smalls_sb")
            src_sb = cp.tile([P, SC], I32, tag="src_sb")
            srcf_sb = cp.tile([P, SC], F32, tag="srcf_sb")
            ident_sb = cp.tile([P, P], F32, tag="ident_sb")
            identb_sb = cp.tile([P, P], BF16, tag="identb_sb")
            srcrow_sb = cp.tile([P, TS], F32, tag="srcrow_sb")
            invcntrow_sb = cp.tile([P, TS], F32, tag="invcntrow_sb")
            Dm_sb = cp.tile([P, SC, TS], BF16, tag="Dm_sb")
            cnt_sb = cp.tile([P, SC], F32, tag="cnt_sb")
            invcnt_sb = cp.tile([P, SC], F32, tag="invcnt_sb")
            qTb_sb = cp.tile([P, KC, TQH], BF16, tag="qTb_sb")
            kTb_sb = cp.tile([P, KC, TS], BF16, tag="kTb_sb")
            attn_sb = cp.tile([P, MI, TS], BF16, tag="attn_sb")
            attnT_sb = cp.tile([P, SC, TQH], BF16, tag="attnT_sb")
            e_sb = cp.tile([P, MI, TS], F32, tag="e_sb")
            sume_sb = cp.tile([P, MI], F32, tag="sume_sb")
            denom_sb = cp.tile([P, MI], F32, tag="denom_sb")
            rden_sb = cp.tile([P, MI], F32, tag="rden_sb")
            s2_sb = cp.tile([P, MI], F32, tag="s2_sb")

            def wqf_c(kc, mc):
                o = PK1_WQF + kc * D + mc * P
                return pk1_sb[:, o : o + P]

            def decTb_c(kc):
                o = PK1_DEC + kc * TQH
                return pk1_sb[:, o : o + TQH]

            def wkb_c(kc, mc):
                o = PK2_WKB + kc * D + mc * P
                return pk2_sb[:, o : o + P]

            def encb_c(kc):
                o = PK2_ENC + kc * D
                return pk2_sb[:, o : o + D]

            def p1cb_v(mi):
                o = PK2_P1C + mi * TS
                return pk2_sb[:, o : o + TS]

            def s1_col(mi):
                return smalls_sb[:, SM_S1 + mi : SM_S1 + mi + 1]

            def w_col(mi):
                return smalls_sb[:, SM_W + mi : SM_W + mi + 1]

            # ---- loads (src rides sync first: tiny, and the Dm build needs
            #      it before the scalar queue's small packets would drain
            #      behind pk1/pk2's 8KB streams; pk loads are split per-kc
            #      chunk so the first Q matmul starts ~4us earlier) ----
            nc.sync.dma_start(
                out=src_sb[:], in_=src[:].rearrange("(c p) n -> p (c n)", p=P)
            )
            nc.scalar.dma_start(out=smalls_sb[:], in_=smalls[:])
            for kc in range(KC):
                nc.sync.dma_start(
                    out=pk1_sb[:, kc * TQH : (kc + 1) * TQH],
                    in_=pk1[:, kc * TQH : (kc + 1) * TQH],
                )
                nc.gpsimd.dma_start(
                    out=pk1_sb[:, PK1_WQF + kc * D : PK1_WQF + (kc + 1) * D],
                    in_=pk1[:, PK1_WQF + kc * D : PK1_WQF + (kc + 1) * D],
                )
            for kc in range(KC):
                nc.sync.dma_start(
                    out=pk2_sb[:, kc * D : (kc + 1) * D],
                    in_=pk2[:, kc * D : (kc + 1) * D],
                )
                nc.gpsimd.dma_start(
                    out=pk2_sb[:, PK2_ENC + kc * D : PK2_ENC + (kc + 1) * D],
                    in_=pk2[:, PK2_ENC + kc * D : PK2_ENC + (kc + 1) * D],
                )
            nc.gpsimd.dma_start(
                out=pk2_sb[:, PK2_P1C:PK2_COLS], in_=pk2[:, PK2_P1C:PK2_COLS]
            )
            nc.vector.tensor_copy(srcf_sb[:], src_sb[:])
            make_identity(nc, ident_sb[:])
            make_identity(nc, identb_sb[:])

            # ---- Q_T = Wqf @ dec.T + bqf  (fcQ folded on host; emitted
            #      first so PE starts the moment pk1 lands) ----
            for mc in range(KC):
                ps = psp.tile([P, TQH], F32, tag="ps")
                for kc in range(KC):
                    nc.tensor.matmul(
                        out=ps[:],
                        lhsT=wqf_c(kc, mc),
                        rhs=decTb_c(kc),
                        start=(kc == 0),
                        stop=(kc == KC - 1),
                    )
                nc.scalar.activation(
                    qTb_sb[:, mc, :], ps[:], AF.Identity,
                    bias=smalls_sb[:, SM_BQF + mc : SM_BQF + mc + 1], scale=1.0,
                )

            # ---- selection matrix Dm, counts (PE transposes slot between
            #      Q and K; Dm itself is only needed ~10us later) ----
            for c in range(SC):
                pt = psp.tile([P, P], F32, tag="ps")
                nc.tensor.transpose(
                    out=pt[:],
                    in_=srcf_sb[:, c : c + 1].to_broadcast([P, P]),
                    identity=ident_sb[:],
                )
                nc.vector.tensor_copy(srcrow_sb[:, c * P : (c + 1) * P], pt[:])
            for a in range(SC):
                nc.vector.tensor_tensor(
                    out=Dm_sb[:, a, :],
                    in0=srcf_sb[:, a : a + 1].to_broadcast([P, TS]),
                    in1=srcrow_sb[:],
                    op=ALU.is_equal,
                )
                nc.vector.tensor_reduce(
                    cnt_sb[:, a : a + 1], Dm_sb[:, a, :], AX.X, ALU.add
                )
            nc.vector.reciprocal(invcnt_sb[:], cnt_sb[:])

            # ---- K_T = Wk @ enc.T + bk ----
            for mc in range(KC):
                ps = psp.tile([P, TS], F32, tag="ps")
                for kc in range(KC):
                    nc.tensor.matmul(
                        out=ps[:],
                        lhsT=wkb_c(kc, mc),
                        rhs=encb_c(kc),
                        start=(kc == 0),
                        stop=(kc == KC - 1),
                    )
                nc.scalar.activation(
                    kTb_sb[:, mc, :], ps[:], AF.Identity,
                    bias=smalls_sb[:, SM_BK + mc : SM_BK + mc + 1], scale=1.0,
                )

            # ---- invcnt row layout (PE transposes; off the critical path,
            #      needed only for g) ----
            for c in range(SC):
                pt = psp.tile([P, P], F32, tag="ps")
                nc.tensor.transpose(
                    out=pt[:],
                    in_=invcnt_sb[:, c : c + 1].to_broadcast([P, P]),
                    identity=ident_sb[:],
                )
                nc.vector.tensor_copy(invcntrow_sb[:, c * P : (c + 1) * P], pt[:])

            p1_v = p1[:].rearrange("(mi p) v -> p mi v", p=P)
            out_v = out[:].rearrange("(mi p) v -> p mi v", p=P)

            # pre-issue every p1 tile load, alternating between the two
            # HWDGE queues: a single queue's reads run ~20 GB/s per DMA
            # engine, two active queues interleave to ~26 (measured); the
            # pool dependency (pin N reuses pin N-13's buffer, freed by
            # blend N-13) paces the tail reads
            pins = []
            for mi in range(MI):
                for vt in range(NVT):
                    i = mi * NVT + vt
                    pin = pinp.tile([P, VT], BF16, tag="pin")
                    # odd pins ride gpsimd's SWDGE queue: a second active
                    # queue lifts reads from ~20 to ~26 GB/s per DMA engine,
                    # and gpsimd runs no compute, so the issue-pacing waits
                    # that poisoned the scalar engine's ACT stream land on
                    # an otherwise-idle engine
                    eng = nc.gpsimd if i % 2 == 1 else nc.sync
                    eng.dma_start(
                        out=pin[:], in_=p1_v[:, mi, vt * VT : (vt + 1) * VT]
                    )
                    pins.append(pin)

            def blend_tile(mi, vt, out_engine):
                vs = slice(vt * VT, (vt + 1) * VT)
                pin = pins[mi * NVT + vt]
                pout = poutp.tile([P, VT], BF16, tag="pout")
                nc.vector.tensor_scalar(
                    out=pout[:], in0=pin[:],
                    scalar1=s1_col(mi),
                    scalar2=s2_sb[:, mi : mi + 1],
                    op0=ALU.mult, op1=ALU.add,
                )
                out_engine.dma_start(out=out_v[:, mi, vs], in_=pout[:])

            def attn_heads(mi):
                # scores + per-head softmax (no max subtraction: |logit| is a
                # ~N(0,1) sample, exp is safe in fp32); accumulate the sum of
                # per-head softmaxes (the 1/NH head-mean folds into the
                # e = exp(a_comb/NH) scale below)
                for h in range(NH):
                    hc, hp = h // 2, h % 2
                    ps = psp.tile([P, TS], F32, tag="ps")
                    nc.tensor.matmul(
                        out=ps[:],
                        lhsT=qTb_sb[hp * DH : (hp + 1) * DH, hc, mi * P : (mi + 1) * P],
                        rhs=kTb_sb[hp * DH : (hp + 1) * DH, hc, :],
                        start=True,
                        stop=True,
                    )
                    ex = wp.tile([P, TS], BF16, tag="ex")
                    se = wp.tile([P, 1], F32, tag="se")
                    nc.scalar.activation(
                        ex[:], ps[:], AF.Exp,
                        bias=0.0, scale=0.125, accum_out=se[:, 0:1],
                    )
                    r8 = wp.tile([P, 1], F32, tag="r8")
                    nc.vector.reciprocal(r8[:], se[:, 0:1])
                    if h == 0:
                        nc.vector.tensor_scalar_mul(attn_sb[:, mi, :], ex[:], r8[:, 0:1])
                    else:
                        nc.vector.scalar_tensor_tensor(
                            out=attn_sb[:, mi, :],
                            in0=ex[:],
                            scalar=r8[:, 0:1],
                            in1=attn_sb[:, mi, :],
                            op0=ALU.mult,
                            op1=ALU.add,
                        )

            def attn_tail(mi):
                # attn_T via PE transpose (for the a_comb contraction)
                for sc in range(SC):
                    pt = psp.tile([P, P], BF16, tag="ps")
                    nc.tensor.transpose(
                        out=pt[:],
                        in_=attn_sb[:, mi, sc * P : (sc + 1) * P],
                        identity=identb_sb[:],
                    )
                    nc.vector.tensor_copy(attnT_sb[:, sc, mi * P : (mi + 1) * P], pt[:])

                # a_comb = attn @ Dm ; e = exp(a_comb/NH) ; denom ; s2
                ps = psp.tile([P, TS], F32, tag="ps")
                for c in range(SC):
                    nc.tensor.matmul(
                        out=ps[:],
                        lhsT=attnT_sb[:, c, mi * P : (mi + 1) * P],
                        rhs=Dm_sb[:, c, :],
                        start=(c == 0),
                        stop=(c == SC - 1),
                    )
                nc.scalar.activation(
                    e_sb[:, mi, :], ps[:], AF.Exp, bias=0.0, scale=1.0 / NH
                )
                g = wp.tile([P, TS], F32, tag="g")
                nc.vector.scalar_tensor_tensor(
                    out=g[:],
                    in0=e_sb[:, mi, :],
                    scalar=-1.0,
                    in1=invcntrow_sb[:],
                    op0=ALU.add,
                    op1=ALU.mult,
                )
                nc.vector.tensor_reduce(sume_sb[:, mi : mi + 1], g[:], AX.X, ALU.add)
                nc.vector.tensor_scalar_add(
                    denom_sb[:, mi : mi + 1], sume_sb[:, mi : mi + 1], float(V)
                )
                nc.vector.reciprocal(rden_sb[:, mi : mi + 1], denom_sb[:, mi : mi + 1])
                nc.vector.tensor_tensor(
                    out=s2_sb[:, mi : mi + 1], in0=w_col(mi),
                    in1=rden_sb[:, mi : mi + 1], op=ALU.mult,
                )

            # both chains fully before any blend, with the head loops
            # interleaved ahead of the tails: chain1's exps run on ACT right
            # behind chain0's instead of waiting for chain0's whole tail,
            # landing s2[0] and s2[1] within ~2us of each other.
            attn_heads(0)
            attn_heads(1)
            attn_tail(0)
            attn_tail(1)

            # ---- fix columns early (off the kernel tail):
            #      fix = s1*p1c + s2*e (bf16 side output) ----
            for mi in range(MI):
                t2 = wp.tile([P, TS], F32, tag="fix_t2")
                nc.vector.tensor_scalar_mul(t2[:], e_sb[:, mi, :], s2_sb[:, mi : mi + 1])
                fb = wp.tile([P, TS], BF16, tag="fix_fb")
                nc.vector.scalar_tensor_tensor(
                    out=fb[:], in0=p1cb_v(mi), scalar=s1_col(mi), in1=t2[:],
                    op0=ALU.mult, op1=ALU.add,
                )
                nc.scalar.dma_start(
                    out=fixc[:].rearrange("(mi p) s -> p mi s", p=P)[:, mi, :],
                    in_=fb[:],
                )

            # force the blends after every chain op in each engine's stream:
            # the scheduler otherwise interleaves them ahead of chain1's DVE
            # tail, and a pout-stalled blend then blocks s2[1] by ~15us
            for mi in range(MI):
                for vt in range(NVT):
                    with tc.tile_wait_until(1.0 + 0.01 * (mi * NVT + vt)):
                        blend_tile(mi, vt, nc.scalar)

    nc.finalize()
    return nc


def _get_nc():
    global _NC_CACHE
    if _NC_CACHE is None:
        _NC_CACHE = build_nc()
    return _NC_CACHE


def _pack_kc(m):
    # [D, cols] -> [P, KC*cols] with row r = kc*P + p at cols [kc*cols ...)
    d, cols = m.shape
    return np.ascontiguousarray(
        m.reshape(KC, P, cols).transpose(1, 0, 2).reshape(P, KC * cols)
    )


def _pack_mi(m):
    # [TQH, cols] -> [P, MI*cols]
    _, cols = m.shape
    return np.ascontiguousarray(
        m.reshape(MI, P, cols).transpose(1, 0, 2).reshape(P, MI * cols)
    )


def kernel(**inputs) -> np.ndarray:
    dec = np.asarray(inputs["dec_output"], dtype=np.float32)  # [4, 512, 512]
    enc = np.asarray(inputs["enc_output"], dtype=np.float32)  # [4, 512, 512]
    src = np.asarray(inputs["src"]).astype(np.int32)  # [4, 512]
    p1 = np.asarray(inputs["p1"], dtype=np.float32)  # [4, 512, 32000]
    WfcQ = np.asarray(inputs["WfcQ"], dtype=np.float32)
    bfcQ = np.asarray(inputs["bfcQ"], dtype=np.float32)
    Wq = np.asarray(inputs["Wq"], dtype=np.float32)
    bq = np.asarray(inputs["bq"], dtype=np.float32)
    Wk = np.asarray(inputs["Wk"], dtype=np.float32)
    bk = np.asarray(inputs["bk"], dtype=np.float32)
    Wfcw = np.asarray(inputs["Wfcw"], dtype=np.float32)
    bfcw = np.asarray(inputs["bfcw"], dtype=np.float32)

    B, TQ, _ = dec.shape
    n_cores = 8

    import ml_dtypes

    bf16 = ml_dtypes.bfloat16

    # fold fcQ into the query projection; gate w computed on host in fp32
    WqfT = np.ascontiguousarray((Wq @ WfcQ).T.astype(bf16))
    bqf = Wq @ bfcQ + bq
    w_all = 1.0 / (1.0 + np.exp(-(dec @ Wfcw[0] + bfcw[0])))  # [4, 512] f32

    wqf_pk = _pack_kc(WqfT)
    wkb_pk = _pack_kc(np.ascontiguousarray(Wk.T.astype(bf16)))
    bqf_pk = bqf.reshape(KC, P).T
    bk_pk = bk.reshape(KC, P).T

    in_maps = []
    for core in range(n_cores):
        b, qh = core // 2, core % 2
        qs = slice(qh * TQH, (qh + 1) * TQH)
        p1_slab = p1[b, qs, :]
        p1c = p1_slab[:, src[b]]  # [TQH, TS] f32 host gather
        pk1 = np.concatenate(
            [_pack_kc(np.ascontiguousarray(dec[b].T[:, qs].astype(bf16))), wqf_pk],
            axis=1,
        )
        pk2 = np.concatenate(
            [
                wkb_pk,
                _pack_kc(np.ascontiguousarray(enc[b].T.astype(bf16))),
                _pack_mi(p1c.astype(bf16)),
            ],
            axis=1,
        )
        w_core = w_all[b, qs].reshape(MI, P).T  # [P, MI]
        smalls = np.concatenate(
            [bqf_pk, bk_pk, w_core, 1.0 - w_core], axis=1
        ).astype(np.float32)
        in_maps.append(
            {
                "pk1": np.ascontiguousarray(pk1),
                "pk2": np.ascontiguousarray(pk2),
                "smalls": np.ascontiguousarray(smalls),
                "src": np.ascontiguousarray(src[b].reshape(TS, 1)),
                "p1": np.ascontiguousarray(p1_slab.astype(bf16)),
            }
        )

    nc = _get_nc()
    res = run_bass_kernel_spmd(nc, in_maps, core_ids=list(range(n_cores)))
    global _LAST_RESULTS
    _LAST_RESULTS = res

    out = np.empty((B, TQ, V), dtype=np.float32)
    for core in range(n_cores):
        b, qh = core // 2, core % 2
        qs = slice(qh * TQH, (qh + 1) * TQH)
        out[b, qs, :] = np.asarray(res.results[core]["out"]).astype(np.float32)
        # place the corrected source-token columns (duplicates carry
        # identical values, so overwrite order does not matter)
        out[b, qs, :][:, src[b]] = np.asarray(res.results[core]["fixc"]).astype(
            np.float32
        )
    return out
